# revision 36
# baseline (speedup 1.0000x reference)
"""Trainium2 Bass kernel for gated inception-conv attention (8 cores, seq-parallel).

Shapes (hardcoded): q_data/k_data (1,8,1024,512) f32, bias (1,8,1024,1024) f32,
k_mask (1,8,1024) i32, Wq/Wk/Wv/Wg (512,512), bg (512), Wo (512,512), bo (512),
qcw/kcw/vcw (64,1,3), qcb/kcb/vcb (64).  Output (1,8,1024,512) f32.

Strategy: one sequence per NeuronCore; all-bf16 compute with f32 PSUM
accumulation.  Scores kept transposed (L_k on partitions) so the softmax sum
rides the AV matmul (ones column on V).  The pair bias enters as
host-precomputed exp(bias)^T bf16 multiplied into exp(qk) on DVE/GpSimd;
k-mask folds into the exp's per-partition bias port.  Inputs are transposed
by XBAR DMA-transpose; the depthwise inception conv is 5 shifted elementwise
ops per tile (identity taps on cores 0-3 keep the SPMD program uniform).
"""

import os
import sys

sys.path.insert(0, "/opt/trn_rl_repo")

import numpy as np
import ml_dtypes

import concourse.bass as bass
import concourse.mybir as mybir
from concourse import bacc, tile
from concourse.bass_utils import run_bass_kernel_spmd

BF16 = ml_dtypes.bfloat16
F32 = mybir.dt.float32
BF16D = mybir.dt.bfloat16
FP = mybir.ActivationFunctionType
MULT = mybir.AluOpType.mult
ADD = mybir.AluOpType.add

H, D, L, C = 8, 64, 1024, 512
KD = VD = 512
NCORES = 8
MASK_NEG = -100000.0

# fraction of ebias mults routed to gpsimd: every GP_EVERY-th op
GP_EVERY = int(os.environ.get("K_GP_EVERY", "4"))


LDW_OPT = os.environ.get("K_LDWOPT", "0") == "1"
if LDW_OPT:
    import concourse.bass_utils as _bu

    if not getattr(_bu, "_ldwopt_patched", False):
        _orig_run_command = _bu.run_command

        def _patched_run_command(argv, **kw):
            argv = [
                "--enable-ldw-opt=true" if a == "--enable-ldw-opt=false" else a
                for a in argv
            ]
            return _orig_run_command(argv, **kw)

        _bu.run_command = _patched_run_command
        _bu._ldwopt_patched = True


def build():
    nc = bacc.Bacc(
        "TRN2",
        target_bir_lowering=False,
        debug=False,
        enable_asserts=False,
    )

    # ---- DRAM I/O (host pre-laid-out) ----
    qd = nc.dram_tensor("qd", [128, 4, L], BF16D, kind="ExternalInput").ap()
    kd = nc.dram_tensor("kd", [128, 4, L], BF16D, kind="ExternalInput").ap()
    # exp(bias)^T, bf16, fused head pairs: [hp, kc, p, h01*1024 + q]
    ebias = nc.dram_tensor("ebias", [4, 8, 128, 2048], BF16D, kind="ExternalInput").ap()
    maskneg = nc.dram_tensor("maskneg", [128, 8], F32, kind="ExternalInput").ap()
    wq = nc.dram_tensor("wq", [128, 4, KD], BF16D, kind="ExternalInput").ap()
    wk = nc.dram_tensor("wk", [128, 4, KD], BF16D, kind="ExternalInput").ap()
    wv = nc.dram_tensor("wv", [128, 4, VD], BF16D, kind="ExternalInput").ap()
    wg = nc.dram_tensor("wg", [128, 4, VD], BF16D, kind="ExternalInput").ap()
    wo = nc.dram_tensor("wo", [128, 4, C], BF16D, kind="ExternalInput").ap()
    convw = nc.dram_tensor("convw", [128, 12], F32, kind="ExternalInput").ap()
    bgbo = nc.dram_tensor("bgbo", [128, 8], F32, kind="ExternalInput").ap()
    selc = nc.dram_tensor("selc", [2, 128], F32, kind="ExternalInput").ap()
    identv = nc.dram_tensor("identv", [128, 64], BF16D, kind="ExternalInput").ap()
    out = nc.dram_tensor("out", [C, L], F32, kind="ExternalOutput").ap()

    with tile.TileContext(nc) as tc, nc.allow_low_precision(
        reason="bf16 compute; rel-err budget 2e-2"
    ):
        _body(tc, locals())
    nc.compile()
    return nc


def _body(tc, t):
    nc = tc.nc
    qd, kd, ebias, maskneg = t["qd"], t["kd"], t["ebias"], t["maskneg"]
    wq, wk, wv, wg, wo = t["wq"], t["wk"], t["wv"], t["wg"], t["wo"]
    convw, bgbo, selc, identv, out = (
        t["convw"], t["bgbo"], t["selc"], t["identv"], t["out"],
    )

    with tc.tile_pool(name="const", bufs=1) as const, \
         tc.tile_pool(name="big", bufs=1) as big, \
         tc.tile_pool(name="ep", bufs=3) as ep, \
         tc.tile_pool(name="ebp", bufs=4) as ebp, \
         tc.tile_pool(name="inw", bufs=1) as inw, \
         tc.tile_pool(name="qkps", bufs=2, space="PSUM") as qkp, \
         tc.tile_pool(name="avps", bufs=2, space="PSUM") as avp:

        # ---- inputs needed first: activations + projection weights ----
        qdT = inw.tile([128, 4, L], BF16D, name="qdT", tag="qdT")
        for ks in range(4):
            nc.sync.dma_start(qdT[:, ks, :], qd[:, ks, :])
        w_sb = {}
        for nm, wd in (("q", wq), ("k", wk), ("v", wv), ("g", wg)):
            w_sb[nm] = inw.tile([128, 4, 512], BF16D, name=f"w{nm}", tag=f"w{nm}")
        nc.sync.dma_start(w_sb["q"][:], wq)
        kdT = inw.tile([128, 4, L], BF16D, name="kdT", tag="kdT")
        for ks in range(4):
            nc.sync.dma_start(kdT[:, ks, :], kd[:, ks, :])
        nc.sync.dma_start(w_sb["k"][:], wk)
        nc.sync.dma_start(w_sb["v"][:], wv)
        nc.sync.dma_start(w_sb["g"][:], wg)

        # ---- other constants ----
        convw_sb = const.tile([128, 12], F32, name="convw", tag="convw")
        nc.sync.dma_start(convw_sb[:], convw)
        identv_sb = const.tile([128, 64], BF16D, name="identv", tag="identv")
        nc.sync.dma_start(identv_sb[:], identv)
        maskneg_sb = const.tile([128, 8], F32, name="maskneg", tag="maskneg")
        nc.sync.dma_start(maskneg_sb[:], maskneg)
        bgbo_sb = const.tile([128, 8], F32, name="bgbo", tag="bgbo")
        nc.sync.dma_start(bgbo_sb[:], bgbo)
        selc_sb = const.tile([2, 128], F32, name="selc", tag="selc")
        nc.sync.dma_start(selc_sb[:], selc)
        wo_sb = const.tile([128, 4, C], BF16D, name="wo", tag="wo")
        nc.sync.dma_start(wo_sb[:], wo)

        # ---- persistent big tensors ----
        qc_t = [big.tile([128, L], BF16D, name=f"qc{c}", tag=f"qc{c}") for c in range(4)]
        kc_t = [big.tile([128, L], BF16D, name=f"kc{c}", tag=f"kc{c}") for c in range(4)]
        vnat = big.tile([128, H, 8, D + 1], BF16D, name="vnat", tag="vnat")
        nc.vector.memset(vnat[:, :, :, D : D + 1], 2.0)
        gT = big.tile([128, 4, L], BF16D, name="gT", tag="gT")

        # ====== P2-P4 per chunk: q/k/v projections -> conv -> v-transpose ===
        rhsT = {"q": qdT, "k": kdT, "v": kdT}
        cw = {"q": 0, "k": 4, "v": 8}
        with tc.tile_pool(name="pad", bufs=2) as padp, \
             tc.tile_pool(name="cvtmp", bufs=2) as cvp, \
             tc.tile_pool(name="vc", bufs=2) as vcp:
            for c in range(4):
                pads = {}
                for nm in ("q", "k", "v"):
                    pads[nm] = padp.tile(
                        [128, L + 2], BF16D, name=f"pad{nm}", tag=f"pad{nm}"
                    )
                    nc.vector.memset(pads[nm][:, 0:1], 0.0)
                    nc.vector.memset(pads[nm][:, L + 1 : L + 2], 0.0)
                    for q2 in range(2):
                        ps = qkp.tile([128, 512], F32, name="projps", tag="qk")
                        for ks in range(4):
                            nc.tensor.matmul(
                                ps[:],
                                w_sb[nm][:, ks, c * 128 : (c + 1) * 128],
                                rhsT[nm][:, ks, q2 * 512 : (q2 + 1) * 512],
                                start=(ks == 0),
                                stop=(ks == 3),
                            )
                        nc.scalar.copy(
                            pads[nm][:, 1 + q2 * 512 : 1 + (q2 + 1) * 512], ps[:]
                        )
                # depthwise conv (shift-adds: q/k on DVE, v on gpsimd)
                vc = vcp.tile([128, L], BF16D, name="vc", tag="vc")
                dstc = {"q": qc_t[c], "k": kc_t[c], "v": vc}
                for nm in ("q", "k", "v"):
                    base = cw[nm]
                    w0 = convw_sb[:, base : base + 1]
                    w1 = convw_sb[:, base + 1 : base + 2]
                    w2 = convw_sb[:, base + 2 : base + 3]
                    bb = convw_sb[:, base + 3 : base + 4]
                    x = pads[nm]
                    y = dstc[nm]
                    eng = nc.vector
                    nc.vector.tensor_scalar(y[:], x[:, 1 : L + 1], w1, bb, MULT, ADD)
                    tm = cvp.tile([128, L], BF16D, name="cvtmp", tag="cvtmp")
                    nc.vector.tensor_scalar_mul(tm[:], x[:, 0:L], w0)
                    eng.tensor_tensor(y[:], y[:], tm[:], ADD)
                    tm2 = cvp.tile([128, L], BF16D, name="cvtmp2", tag="cvtmp")
                    nc.vector.tensor_scalar_mul(tm2[:], x[:, 2 : L + 2], w2)
                    eng.tensor_tensor(y[:], y[:], tm2[:], ADD)
                # v -> natural layout, heads 2c/2c+1 row-packed on the PE
                pss = [
                    avp.tile([128, 512], BF16D, name="vtps", tag="av")
                    for _ in range(2)
                ]
                for b in range(8):
                    for h01 in range(2):
                        ph = h01 * 64
                        nc.tensor.transpose(
                            pss[h01][:, b * 64 : (b + 1) * 64],
                            vc[ph : ph + 64, b * 128 : (b + 1) * 128],
                            identv_sb[ph : ph + 64, :],
                        )
                for h01 in range(2):
                    nc.vector.tensor_copy(
                        vnat[:, 2 * c + h01, :, 0:D],
                        pss[h01].rearrange("p (a b) -> p a b", b=64),
                    )

        # ======================= P5: attention ==============================
        with tc.tile_pool(name="late", bufs=1) as late, \
             tc.tile_pool(name="stg", bufs=2) as stg, \
             tc.tile_pool(name="gf2", bufs=2) as gfp2:
            oT_all = late.tile([128, 4, L], F32, name="oT", tag="oT")
            og_all = late.tile([128, 4, L], BF16D, name="oga", tag="oga")
            Dp = late.tile([2, 4, L], F32, name="Dp", tag="Dp")
            nmul = 0
            for hp in range(4):
                av = [
                    avp.tile([65, L], F32, name="av", tag="av")
                    for _ in range(2)
                ]
                for blk in range(4):
                    eT = [
                        ep.tile([128, 2, L], BF16D, name=f"eT{h01}", tag=f"eT{h01}")
                        for h01 in range(2)
                    ]
                    for kk in range(2):
                        kc = blk * 2 + kk
                        qks = []
                        for h01 in range(2):
                            ph = h01 * 64
                            qk = qkp.tile([128, L], F32, name="qk", tag="qk")
                            qks.append(qk)
                            for q2 in range(2):
                                nc.tensor.matmul(
                                    qk[:, q2 * 512 : (q2 + 1) * 512],
                                    kc_t[hp][ph : ph + 64, kc * 128 : (kc + 1) * 128],
                                    qc_t[hp][ph : ph + 64, q2 * 512 : (q2 + 1) * 512],
                                    start=True,
                                    stop=True,
                                )
                        for h01 in range(2):
                            nc.scalar.activation(
                                eT[h01][:, kk, :], qks[h01][:], FP.Exp,
                                bias=maskneg_sb[:, kc : kc + 1],
                            )
                    for h01 in range(2):
                        eb = ebp.tile([128, 2, L], BF16D, name="ebst", tag="ebst")
                        nc.sync.dma_start(
                            eb[:],
                            ebias[
                                hp, blk * 2 : blk * 2 + 2, :,
                                h01 * 1024 : (h01 + 1) * 1024,
                            ].rearrange("kc p q -> p kc q"),
                        )
                        nc.vector.tensor_tensor(eT[h01][:], eT[h01][:], eb[:], MULT)
                        nmul += 1
                    for kk in range(2):
                        kc = blk * 2 + kk
                        for h01 in range(2):
                            for q2 in range(2):
                                nc.tensor.matmul(
                                    av[h01][:, q2 * 512 : (q2 + 1) * 512],
                                    vnat[:, 2 * hp + h01, kc, :],
                                    eT[h01][:, kk, q2 * 512 : (q2 + 1) * 512],
                                    start=(kc == 0),
                                    stop=(kc == 7),
                                )
                for h01 in range(2):
                    h = 2 * hp + h01
                    st = stg.tile([65, L], F32, name="st", tag="st")
                    nc.vector.tensor_copy(st[:], av[h01][:])
                    nc.sync.dma_start(
                        oT_all[(h % 2) * 64 : (h % 2) * 64 + 64, h // 2, :],
                        st[0:64, :],
                    )
                    nc.sync.dma_start(Dp[h01 : h01 + 1, hp, :], st[64:65, :])
                nc.vector.reciprocal_approx_fast(Dp[:, hp, :], Dp[:, hp, :])
                for q2 in range(2):
                    gps = qkp.tile([128, 512], F32, name="gps", tag="qk")
                    for ks in range(4):
                        nc.tensor.matmul(
                            gps[:],
                            w_sb["g"][:, ks, hp * 128 : (hp + 1) * 128],
                            qdT[:, ks, q2 * 512 : (q2 + 1) * 512],
                            start=(ks == 0),
                            stop=(ks == 3),
                        )
                    nc.scalar.activation(
                        gT[:, hp, q2 * 512 : (q2 + 1) * 512], gps[:], FP.Tanh,
                        bias=bgbo_sb[:, hp : hp + 1], scale=0.5,
                    )
                    rt = qkp.tile([128, 512], F32, name="rt", tag="qk")
                    nc.tensor.matmul(
                        rt[:],
                        selc_sb[:],
                        Dp[:, hp, q2 * 512 : (q2 + 1) * 512],
                        start=True,
                        stop=True,
                    )
                    gp2 = gfp2.tile([128, 512], BF16D, name="gp2", tag="gf2")
                    nc.vector.tensor_scalar_add(
                        gp2[:], gT[:, hp, q2 * 512 : (q2 + 1) * 512], 1.0
                    )
                    gf = gfp2.tile([128, 512], BF16D, name="gfb", tag="gf2")
                    nc.vector.tensor_tensor(gf[:], gp2[:], rt[:], MULT)
                    nc.vector.tensor_tensor(
                        og_all[:, hp, q2 * 512 : (q2 + 1) * 512],
                        oT_all[:, hp, q2 * 512 : (q2 + 1) * 512],
                        gf[:],
                        MULT,
                    )

            # ====== P6: out-proj only (gate/normalize done in P5) =======
            with tc.tile_pool(name="og6", bufs=1) as og6:
                out_sb = og6.tile([128, 4, L], F32, name="outsb", tag="outsb")
                for mc in range(4):
                    for q2 in range(2):
                        pso = avp.tile([128, 512], F32, name="ops", tag="av")
                        for ks in range(4):
                            nc.tensor.matmul(
                                pso[:],
                                wo_sb[:, ks, mc * 128 : (mc + 1) * 128],
                                og_all[:, ks, q2 * 512 : (q2 + 1) * 512],
                                start=(ks == 0),
                                stop=(ks == 3),
                            )
                        nc.vector.tensor_scalar_add(
                            out_sb[:, mc, q2 * 512 : (q2 + 1) * 512],
                            pso[:],
                            bgbo_sb[:, 4 + mc : 5 + mc],
                        )
                nc.sync.dma_start(
                    out.rearrange("(mc p) l -> p mc l", p=128), out_sb[:]
                )


# ---------------------------------------------------------------------------
# host side
# ---------------------------------------------------------------------------
_NC = None


def _get_nc():
    global _NC
    if _NC is None:
        _NC = build()
    return _NC


def _chunked(w):
    """(512, N) -> (128, 4, N) with row r at [r % 128, r // 128]."""
    n = np.asarray(w).shape[1]
    return np.ascontiguousarray(
        np.asarray(w, np.float32).reshape(4, 128, n).transpose(1, 0, 2)
    ).astype(BF16)


def _ctrans(x):
    """(L, C) -> (128, 4, L) bf16 with channel r at [r % 128, r // 128]."""
    xT = np.asarray(x, np.float32).T  # (C, L)
    return np.ascontiguousarray(
        xT.reshape(4, 128, L).transpose(1, 0, 2)
    ).astype(BF16)


def _prep_inmaps(q_data, k_data, bias, k_mask, Wq, Wk, Wv, Wg, bg, Wo, bo,
                 qcw, qcb, kcw, kcb, vcw, vcb):
    f32 = np.float32
    # exp(bias)^T -> (H, 8, 128, L) bf16: [h, kc, p, q] = exp(bias[h, q, kc*128+p])
    ebT = np.exp(np.asarray(bias[0], f32)).transpose(0, 2, 1)  # (h, k, q)
    ebias = np.ascontiguousarray(
        ebT.reshape(4, 2, 8, 128, L).transpose(0, 2, 3, 1, 4).reshape(4, 8, 128, 2 * L)
    ).astype(BF16)

    wq_a, wk_a, wv_a, wg_a, wo_a = (_chunked(w) for w in (Wq, Wk, Wv, Wg, Wo))
    bgbo = np.zeros((128, 8), f32)
    bgbo[:, 0:4] = 0.5 * np.asarray(bg, f32).reshape(4, 128).T
    bgbo[:, 4:8] = np.asarray(bo, f32).reshape(4, 128).T

    selc = np.zeros((2, 128), f32)
    for m in range(128):
        selc[m // 64, m] = 1.0

    identv = np.zeros((128, 64), f32)
    identv[np.arange(128), np.arange(128) % 64] = 1.0
    identv = identv.astype(BF16)

    # conv taps per core: identity for seqs 0-3, real for 4-7; q scaled D^-0.5
    scale = 1.0 / np.sqrt(D)
    dd = np.arange(128) % 64

    def taps(w3, b1, use_real, s):
        cwc = np.zeros((128, 4), f32)
        if use_real:
            cwc[:, 0:3] = np.asarray(w3, f32)[dd, 0, :] * s
            cwc[:, 3] = np.asarray(b1, f32)[dd] * s
        else:
            cwc[:, 1] = s
        return cwc

    in_maps = []
    for s in range(NCORES):
        real = s >= 4
        cwm = np.concatenate(
            [
                taps(qcw, qcb, real, scale),
                taps(kcw, kcb, real, 1.0),
                taps(vcw, vcb, real, 1.0),
            ],
            axis=1,
        ).astype(f32)
        mk = np.asarray(k_mask[0, s], np.int32).reshape(8, 128).T  # (128, 8)
        maskneg = np.where(mk != 0, 0.0, MASK_NEG).astype(f32)
        in_maps.append(
            {
                "qd": _ctrans(q_data[0, s]),
                "kd": _ctrans(k_data[0, s]),
                "ebias": ebias,
                "maskneg": maskneg,
                "wq": wq_a, "wk": wk_a, "wv": wv_a, "wg": wg_a, "wo": wo_a,
                "convw": cwm,
                "bgbo": bgbo,
                "selc": selc,
                "identv": identv,
            }
        )
    return in_maps


def run(in_maps, trace=False):
    nc = _get_nc()
    return run_bass_kernel_spmd(
        nc, in_maps, core_ids=list(range(NCORES)), trace=trace
    )


def kernel(**inputs):
    in_maps = _prep_inmaps(**inputs)
    res = run(in_maps)
    outp = np.empty((1, NCORES, L, C), np.float32)
    for s in range(NCORES):
        outp[0, s] = res.results[s]["out"].T
    return outp



# revision 37
# speedup vs baseline: 1.2820x; 1.2820x over previous
"""Trainium2 Bass kernel for gated inception-conv attention (8 cores, seq-parallel).

Shapes (hardcoded): q_data/k_data (1,8,1024,512) f32, bias (1,8,1024,1024) f32,
k_mask (1,8,1024) i32, Wq/Wk/Wv/Wg (512,512), bg (512), Wo (512,512), bo (512),
qcw/kcw/vcw (64,1,3), qcb/kcb/vcb (64).  Output (1,8,1024,512) f32.

Strategy: one sequence per NeuronCore; all-bf16 compute with f32 PSUM
accumulation.  Scores kept transposed (L_k on partitions) so the softmax sum
rides the AV matmul (ones column on V).  The pair bias enters as
host-precomputed exp(bias)^T bf16 multiplied into exp(qk) on DVE/GpSimd;
k-mask folds into the exp's per-partition bias port.  Inputs are transposed
by XBAR DMA-transpose; the depthwise inception conv is 5 shifted elementwise
ops per tile (identity taps on cores 0-3 keep the SPMD program uniform).
"""

import os
import sys

sys.path.insert(0, "/opt/trn_rl_repo")

import numpy as np
import ml_dtypes

import concourse.bass as bass
import concourse.mybir as mybir
from concourse import bacc, tile
from concourse.bass_utils import run_bass_kernel_spmd

BF16 = ml_dtypes.bfloat16
F32 = mybir.dt.float32
BF16D = mybir.dt.bfloat16
FP = mybir.ActivationFunctionType
MULT = mybir.AluOpType.mult
ADD = mybir.AluOpType.add

H, D, L, C = 8, 64, 1024, 512
KD = VD = 512
NCORES = 8
MASK_NEG = -100000.0

# fraction of ebias mults routed to gpsimd: every GP_EVERY-th op
GP_EVERY = int(os.environ.get("K_GP_EVERY", "4"))


LDW_OPT = os.environ.get("K_LDWOPT", "0") == "1"
if LDW_OPT:
    import concourse.bass_utils as _bu

    if not getattr(_bu, "_ldwopt_patched", False):
        _orig_run_command = _bu.run_command

        def _patched_run_command(argv, **kw):
            argv = [
                "--enable-ldw-opt=true" if a == "--enable-ldw-opt=false" else a
                for a in argv
            ]
            return _orig_run_command(argv, **kw)

        _bu.run_command = _patched_run_command
        _bu._ldwopt_patched = True


def build():
    nc = bacc.Bacc(
        "TRN2",
        target_bir_lowering=False,
        debug=False,
        enable_asserts=False,
    )

    # ---- DRAM I/O (host pre-laid-out) ----
    qd = nc.dram_tensor("qd", [128, 4, L], BF16D, kind="ExternalInput").ap()
    kd = nc.dram_tensor("kd", [128, 4, L], BF16D, kind="ExternalInput").ap()
    # exp(bias)^T, bf16: [hp, h01, p, kc*1024 + q]
    ebias = nc.dram_tensor("ebias", [4, 2, 128, 8 * 1024], BF16D, kind="ExternalInput").ap()
    maskneg = nc.dram_tensor("maskneg", [128, 8], F32, kind="ExternalInput").ap()
    wq = nc.dram_tensor("wq", [128, 4, KD], BF16D, kind="ExternalInput").ap()
    wk = nc.dram_tensor("wk", [128, 4, KD], BF16D, kind="ExternalInput").ap()
    wv = nc.dram_tensor("wv", [128, 4, VD], BF16D, kind="ExternalInput").ap()
    wg = nc.dram_tensor("wg", [128, 4, VD], BF16D, kind="ExternalInput").ap()
    wo = nc.dram_tensor("wo", [128, 4, C], BF16D, kind="ExternalInput").ap()
    convw = nc.dram_tensor("convw", [128, 12], F32, kind="ExternalInput").ap()
    bgbo = nc.dram_tensor("bgbo", [128, 8], F32, kind="ExternalInput").ap()
    selc = nc.dram_tensor("selc", [2, 128], BF16D, kind="ExternalInput").ap()
    identv = nc.dram_tensor("identv", [128, 64], BF16D, kind="ExternalInput").ap()
    out = nc.dram_tensor("out", [C, L], F32, kind="ExternalOutput").ap()

    with tile.TileContext(nc) as tc, nc.allow_low_precision(
        reason="bf16 compute; rel-err budget 2e-2"
    ):
        _body(tc, locals())
    nc.compile()
    return nc


def _body(tc, t):
    nc = tc.nc
    qd, kd, ebias, maskneg = t["qd"], t["kd"], t["ebias"], t["maskneg"]
    wq, wk, wv, wg, wo = t["wq"], t["wk"], t["wv"], t["wg"], t["wo"]
    convw, bgbo, selc, identv, out = (
        t["convw"], t["bgbo"], t["selc"], t["identv"], t["out"],
    )

    with tc.tile_pool(name="const", bufs=1) as const, \
         tc.tile_pool(name="big", bufs=1) as big, \
         tc.tile_pool(name="ep", bufs=3) as ep, \
         tc.tile_pool(name="ebp", bufs=4) as ebp, \
         tc.tile_pool(name="inw", bufs=1) as inw, \
         tc.tile_pool(name="qkps", bufs=2, space="PSUM") as qkp, \
         tc.tile_pool(name="avps", bufs=2, space="PSUM") as avp:

        # ---- inputs needed first: activations + projection weights ----
        qdT = inw.tile([128, 4, L], BF16D, name="qdT", tag="qdT")
        for ks in range(4):
            nc.sync.dma_start(qdT[:, ks, :], qd[:, ks, :])
        w_sb = {}
        for nm, wd in (("q", wq), ("k", wk), ("v", wv), ("g", wg)):
            w_sb[nm] = inw.tile([128, 4, 512], BF16D, name=f"w{nm}", tag=f"w{nm}")
        nc.sync.dma_start(w_sb["q"][:], wq)
        kdT = inw.tile([128, 4, L], BF16D, name="kdT", tag="kdT")
        for ks in range(4):
            nc.sync.dma_start(kdT[:, ks, :], kd[:, ks, :])
        nc.sync.dma_start(w_sb["k"][:], wk)
        nc.sync.dma_start(w_sb["v"][:], wv)
        nc.sync.dma_start(w_sb["g"][:], wg)

        # ---- other constants ----
        convw_sb = const.tile([128, 12], F32, name="convw", tag="convw")
        nc.sync.dma_start(convw_sb[:], convw)
        identv_sb = const.tile([128, 64], BF16D, name="identv", tag="identv")
        nc.sync.dma_start(identv_sb[:], identv)
        maskneg_sb = const.tile([128, 8], F32, name="maskneg", tag="maskneg")
        nc.sync.dma_start(maskneg_sb[:], maskneg)
        bgbo_sb = const.tile([128, 8], F32, name="bgbo", tag="bgbo")
        nc.sync.dma_start(bgbo_sb[:], bgbo)
        selc_sb = const.tile([2, 128], BF16D, name="selc", tag="selc")
        nc.sync.dma_start(selc_sb[:], selc)
        wo_sb = const.tile([128, 4, C], BF16D, name="wo", tag="wo")
        nc.sync.dma_start(wo_sb[:], wo)

        # ---- persistent big tensors ----
        qc_t = [big.tile([128, L], BF16D, name=f"qc{c}", tag=f"qc{c}") for c in range(4)]
        kc_t = [big.tile([128, L], BF16D, name=f"kc{c}", tag=f"kc{c}") for c in range(4)]
        vnat = big.tile([128, H, 8, D + 1], BF16D, name="vnat", tag="vnat")
        nc.vector.memset(vnat[:, :, :, D : D + 1], 1.0)
        gT = big.tile([128, 4, L], BF16D, name="gT", tag="gT")

        # ====== P2-P4 per chunk: q/k/v projections -> conv -> v-transpose ===
        rhsT = {"q": qdT, "k": kdT, "v": kdT}
        cw = {"q": 0, "k": 4, "v": 8}
        with tc.tile_pool(name="pad", bufs=2) as padp, \
             tc.tile_pool(name="cvtmp", bufs=2) as cvp, \
             tc.tile_pool(name="vc", bufs=2) as vcp:
            for c in range(4):
                pads = {}
                for nm in ("q", "k", "v"):
                    pads[nm] = padp.tile(
                        [128, L + 2], BF16D, name=f"pad{nm}", tag=f"pad{nm}"
                    )
                    nc.vector.memset(pads[nm][:, 0:1], 0.0)
                    nc.vector.memset(pads[nm][:, L + 1 : L + 2], 0.0)
                    for q2 in range(2):
                        ps = qkp.tile([128, 512], F32, name="projps", tag="qk")
                        for ks in range(4):
                            nc.tensor.matmul(
                                ps[:],
                                w_sb[nm][:, ks, c * 128 : (c + 1) * 128],
                                rhsT[nm][:, ks, q2 * 512 : (q2 + 1) * 512],
                                start=(ks == 0),
                                stop=(ks == 3),
                            )
                        nc.scalar.copy(
                            pads[nm][:, 1 + q2 * 512 : 1 + (q2 + 1) * 512], ps[:]
                        )
                # depthwise conv (shift-adds: q/k on DVE, v on gpsimd)
                vc = vcp.tile([128, L], BF16D, name="vc", tag="vc")
                dstc = {"q": qc_t[c], "k": kc_t[c], "v": vc}
                for nm in ("q", "k", "v"):
                    base = cw[nm]
                    w0 = convw_sb[:, base : base + 1]
                    w1 = convw_sb[:, base + 1 : base + 2]
                    w2 = convw_sb[:, base + 2 : base + 3]
                    bb = convw_sb[:, base + 3 : base + 4]
                    x = pads[nm]
                    y = dstc[nm]
                    eng = nc.vector
                    nc.vector.tensor_scalar(y[:], x[:, 1 : L + 1], w1, bb, MULT, ADD)
                    tm = cvp.tile([128, L], BF16D, name="cvtmp", tag="cvtmp")
                    nc.vector.tensor_scalar_mul(tm[:], x[:, 0:L], w0)
                    eng.tensor_tensor(y[:], y[:], tm[:], ADD)
                    tm2 = cvp.tile([128, L], BF16D, name="cvtmp2", tag="cvtmp")
                    nc.vector.tensor_scalar_mul(tm2[:], x[:, 2 : L + 2], w2)
                    eng.tensor_tensor(y[:], y[:], tm2[:], ADD)
                # v -> natural layout, heads 2c/2c+1 row-packed on the PE
                pss = [
                    avp.tile([128, 512], BF16D, name="vtps", tag="av")
                    for _ in range(2)
                ]
                for b in range(8):
                    for h01 in range(2):
                        ph = h01 * 64
                        nc.tensor.transpose(
                            pss[h01][:, b * 64 : (b + 1) * 64],
                            vc[ph : ph + 64, b * 128 : (b + 1) * 128],
                            identv_sb[ph : ph + 64, :],
                        )
                for h01 in range(2):
                    nc.vector.tensor_copy(
                        vnat[:, 2 * c + h01, :, 0:D],
                        pss[h01].rearrange("p (a b) -> p a b", b=64),
                    )

        # ======================= P5: attention ==============================
        with tc.tile_pool(name="late", bufs=1) as late, \
             tc.tile_pool(name="stg", bufs=2) as stg:
            oT_all = late.tile([128, 4, L], F32, name="oT", tag="oT")
            Dp = late.tile([2, 4, L], F32, name="Dp", tag="Dp")
            Dpb = late.tile([2, 4, L], BF16D, name="Dpb", tag="Dpb")
            nmul = 0
            for hp in range(4):
                av = [
                    avp.tile([65, L], F32, name="av", tag="av")
                    for _ in range(2)
                ]
                for blk in range(4):
                    eT = [
                        ep.tile([128, 2, L], BF16D, name=f"eT{h01}", tag=f"eT{h01}")
                        for h01 in range(2)
                    ]
                    for kk in range(2):
                        kc = blk * 2 + kk
                        qks = []
                        for h01 in range(2):
                            ph = h01 * 64
                            qk = qkp.tile([128, L], F32, name="qk", tag="qk")
                            qks.append(qk)
                            for q2 in range(2):
                                nc.tensor.matmul(
                                    qk[:, q2 * 512 : (q2 + 1) * 512],
                                    kc_t[hp][ph : ph + 64, kc * 128 : (kc + 1) * 128],
                                    qc_t[hp][ph : ph + 64, q2 * 512 : (q2 + 1) * 512],
                                    start=True,
                                    stop=True,
                                )
                        for h01 in range(2):
                            nc.scalar.activation(
                                eT[h01][:, kk, :], qks[h01][:], FP.Exp,
                                bias=maskneg_sb[:, kc : kc + 1],
                            )
                    for h01 in range(2):
                        eb = ebp.tile([128, 2 * L], BF16D, name="ebst", tag="ebst")
                        nc.sync.dma_start(
                            eb[:],
                            ebias[hp, h01, :, blk * 2048 : (blk + 1) * 2048],
                        )
                        nc.vector.tensor_tensor(
                            eT[h01][:],
                            eT[h01][:],
                            eb.rearrange("p (kk q) -> p kk q", kk=2),
                            MULT,
                        )
                        nmul += 1
                    for kk in range(2):
                        kc = blk * 2 + kk
                        for h01 in range(2):
                            for q2 in range(2):
                                nc.tensor.matmul(
                                    av[h01][:, q2 * 512 : (q2 + 1) * 512],
                                    vnat[:, 2 * hp + h01, kc, :],
                                    eT[h01][:, kk, q2 * 512 : (q2 + 1) * 512],
                                    start=(kc == 0),
                                    stop=(kc == 7),
                                )
                for h01 in range(2):
                    h = 2 * hp + h01
                    st = stg.tile([65, L], F32, name="st", tag="st")
                    nc.vector.tensor_copy(st[:], av[h01][:])
                    nc.sync.dma_start(
                        oT_all[(h % 2) * 64 : (h % 2) * 64 + 64, h // 2, :],
                        st[0:64, :],
                    )
                    nc.sync.dma_start(Dp[h01 : h01 + 1, hp, :], st[64:65, :])
                nc.vector.reciprocal_approx_fast(Dp[:, hp, :], Dp[:, hp, :])
                nc.vector.tensor_copy(Dpb[:, hp, :], Dp[:, hp, :])

            # ====== P6: gate projection + normalize + gate + out-proj =======
            with tc.tile_pool(name="gf", bufs=2) as gfp, \
                 tc.tile_pool(name="og6", bufs=1) as og6:
                og = og6.tile([128, 4, L], BF16D, name="og", tag="og")
                out_sb = og6.tile([128, 4, L], F32, name="outsb", tag="outsb")
                for c in range(4):
                    for q2 in range(2):
                        gps = avp.tile([128, 512], F32, name="gps", tag="av")
                        for ks in range(4):
                            nc.tensor.matmul(
                                gps[:],
                                w_sb["g"][:, ks, c * 128 : (c + 1) * 128],
                                qdT[:, ks, q2 * 512 : (q2 + 1) * 512],
                                start=(ks == 0),
                                stop=(ks == 3),
                            )
                        nc.scalar.activation(
                            gT[:, c, q2 * 512 : (q2 + 1) * 512],
                            gps[:],
                            FP.Sigmoid,
                            bias=bgbo_sb[:, c : c + 1],
                        )
                        rt = qkp.tile([128, 512], F32, name="rt", tag="qk")
                        nc.tensor.matmul(
                            rt[:],
                            selc_sb[:],
                            Dpb[:, c, q2 * 512 : (q2 + 1) * 512],
                            start=True,
                            stop=True,
                        )
                        gf = gfp.tile([128, 512], F32, name="gf", tag="gf")
                        nc.vector.tensor_tensor(
                            gf[:], gT[:, c, q2 * 512 : (q2 + 1) * 512], rt[:], MULT
                        )
                        nc.vector.tensor_tensor(
                            og[:, c, q2 * 512 : (q2 + 1) * 512],
                            oT_all[:, c, q2 * 512 : (q2 + 1) * 512],
                            gf[:],
                            MULT,
                        )
                for mc in range(4):
                    for q2 in range(2):
                        pso = avp.tile([128, 512], F32, name="ops", tag="av")
                        for ks in range(4):
                            nc.tensor.matmul(
                                pso[:],
                                wo_sb[:, ks, mc * 128 : (mc + 1) * 128],
                                og[:, ks, q2 * 512 : (q2 + 1) * 512],
                                start=(ks == 0),
                                stop=(ks == 3),
                            )
                        nc.vector.tensor_scalar_add(
                            out_sb[:, mc, q2 * 512 : (q2 + 1) * 512],
                            pso[:],
                            bgbo_sb[:, 4 + mc : 5 + mc],
                        )
                nc.sync.dma_start(
                    out.rearrange("(mc p) l -> p mc l", p=128), out_sb[:]
                )


# ---------------------------------------------------------------------------
# host side
# ---------------------------------------------------------------------------
_NC = None


def _get_nc():
    global _NC
    if _NC is None:
        _NC = build()
    return _NC


def _chunked(w):
    """(512, N) -> (128, 4, N) with row r at [r % 128, r // 128]."""
    n = np.asarray(w).shape[1]
    return np.ascontiguousarray(
        np.asarray(w, np.float32).reshape(4, 128, n).transpose(1, 0, 2)
    ).astype(BF16)


def _ctrans(x):
    """(L, C) -> (128, 4, L) bf16 with channel r at [r % 128, r // 128]."""
    xT = np.asarray(x, np.float32).T  # (C, L)
    return np.ascontiguousarray(
        xT.reshape(4, 128, L).transpose(1, 0, 2)
    ).astype(BF16)


def _prep_inmaps(q_data, k_data, bias, k_mask, Wq, Wk, Wv, Wg, bg, Wo, bo,
                 qcw, qcb, kcw, kcb, vcw, vcb):
    f32 = np.float32
    # exp(bias)^T -> (H, 8, 128, L) bf16: [h, kc, p, q] = exp(bias[h, q, kc*128+p])
    ebT = np.exp(np.asarray(bias[0], f32)).transpose(0, 2, 1)  # (h, k, q)
    ebias = np.ascontiguousarray(
        ebT.reshape(4, 2, 8, 128, L).transpose(0, 1, 3, 2, 4).reshape(4, 2, 128, 8 * L)
    ).astype(BF16)

    wq_a, wk_a, wv_a, wg_a, wo_a = (_chunked(w) for w in (Wq, Wk, Wv, Wg, Wo))
    bgbo = np.zeros((128, 8), f32)
    bgbo[:, 0:4] = np.asarray(bg, f32).reshape(4, 128).T
    bgbo[:, 4:8] = np.asarray(bo, f32).reshape(4, 128).T

    selc = np.zeros((2, 128), f32)
    for m in range(128):
        selc[m // 64, m] = 1.0
    selc = selc.astype(BF16)

    identv = np.zeros((128, 64), f32)
    identv[np.arange(128), np.arange(128) % 64] = 1.0
    identv = identv.astype(BF16)

    # conv taps per core: identity for seqs 0-3, real for 4-7; q scaled D^-0.5
    scale = 1.0 / np.sqrt(D)
    dd = np.arange(128) % 64

    def taps(w3, b1, use_real, s):
        cwc = np.zeros((128, 4), f32)
        if use_real:
            cwc[:, 0:3] = np.asarray(w3, f32)[dd, 0, :] * s
            cwc[:, 3] = np.asarray(b1, f32)[dd] * s
        else:
            cwc[:, 1] = s
        return cwc

    in_maps = []
    for s in range(NCORES):
        real = s >= 4
        cwm = np.concatenate(
            [
                taps(qcw, qcb, real, scale),
                taps(kcw, kcb, real, 1.0),
                taps(vcw, vcb, real, 1.0),
            ],
            axis=1,
        ).astype(f32)
        mk = np.asarray(k_mask[0, s], np.int32).reshape(8, 128).T  # (128, 8)
        maskneg = np.where(mk != 0, 0.0, MASK_NEG).astype(f32)
        in_maps.append(
            {
                "qd": _ctrans(q_data[0, s]),
                "kd": _ctrans(k_data[0, s]),
                "ebias": ebias,
                "maskneg": maskneg,
                "wq": wq_a, "wk": wk_a, "wv": wv_a, "wg": wg_a, "wo": wo_a,
                "convw": cwm,
                "bgbo": bgbo,
                "selc": selc,
                "identv": identv,
            }
        )
    return in_maps


def run(in_maps, trace=False):
    nc = _get_nc()
    return run_bass_kernel_spmd(
        nc, in_maps, core_ids=list(range(NCORES)), trace=trace
    )


def kernel(**inputs):
    in_maps = _prep_inmaps(**inputs)
    res = run(in_maps)
    outp = np.empty((1, NCORES, L, C), np.float32)
    for s in range(NCORES):
        outp[0, s] = res.results[s]["out"].T
    return outp

